# revision 15
# baseline (speedup 1.0000x reference)
"""Trainium2 Bass kernel for: 3x3 conv (reflect pad) + BatchNorm + LeakyReLU + mask.

Input  x:    (1, 64, 512, 512) f32
       W:    (128, 64, 3, 3)   f32
       gamma/beta/mean/var: (128,) f32
       mask: (1, 128, 512, 512) int32 (0/1)
Output (1, 128, 512, 512) f32

Strategy (8 cores, SPMD):
  - Shard H spatially: core c computes output rows [64c, 64c+64).
  - Even/odd row interleave, single x copy: host reflect-pads x to
    (64, 514, 514); core c takes its 66-row slab and ships it ONCE as a
    [128, 33*514] bf16 image: partitions 0..63 hold channel ci's EVEN local
    rows (pair index p -> row 2p), partitions 64..127 hold the ODD rows
    (p -> row 2p+1). A K=128 matmul at pair offset p then contracts over two
    adjacent image rows at once (two conv dy taps in one slot).
  - Output row y=2t: pair t covers taps dy=0,1; the lone dy=2 tap (even row
    2t+2) runs as a K=64 matmul on PE rows 0-63. Row y=2t+1: pair t+1 covers
    dy=1,2; lone dy=0 (odd row 2t+1) on PE rows 64-127. The two lone streams
    use disjoint PE row groups -> concurrent, so 2 rows cost 9 matmul slots
    (the algorithmic minimum for 9 taps at K=64 on a K=128 array).
  - 4-row groups, weight-major matmul order: consecutive matmuls reuse the
    same stationary tile so the PE can skip redundant LDWEIGHTS.
  - Epilogue: ACT Lrelu(psum*scale+shift) -> bf16, DVE multiply by uint8
    mask; bf16 stores (harness tolerance 2e-2 >> bf16 rounding).
  - DMA: x+masks on the sync HWDGE ring, weights first + stores on the
    scalar HWDGE ring. No SWDGE.
"""

import numpy as np
import ml_dtypes

import concourse.bacc as bacc
import concourse.bass as bass
import concourse.mybir as mybir
import concourse.tile as tile
from concourse.bass_utils import run_bass_kernel_spmd

bf16 = ml_dtypes.bfloat16

N_CORES = 8
C_IN = 64
C_OUT = 128
H = 512
W_IMG = 512
HS = H // N_CORES            # 64 output rows per core
WP = W_IMG + 2               # 514 padded columns
NPAIR = HS // 2 + 1          # 33 even/odd row pairs per core
FREE = NPAIR * WP            # per-partition free elems of the x image
G = 8                        # output rows per mask tile
SG = 4                       # output rows per store tile / PSUM group
LEAK = 0.01
EPS = 1e-5

_CACHE = {}
LAST_RESULTS = None          # BassKernelResults of the last run (for test.py)


def _build_program(hw_lrelu: bool = True) -> bass.Bass:
    """hw_lrelu=True uses the ACT engine's native Lrelu (not implemented in
    CoreSim); False uses an Identity + DVE max(z*a, z) fallback."""
    nc = bacc.Bacc("TRN2", target_bir_lowering=False, debug=False,
                   num_devices=N_CORES)
    f32 = mybir.dt.float32
    bf = mybir.dt.bfloat16
    u8 = mybir.dt.uint8

    xs_d = nc.dram_tensor("xs", [128, FREE], bf, kind="ExternalInput")
    wp_d = nc.dram_tensor("wp", [128, 9 * C_OUT], bf, kind="ExternalInput")
    bn_d = nc.dram_tensor("bn", [C_OUT, 2], f32, kind="ExternalInput")
    mk_d = nc.dram_tensor("msk", [C_OUT, HS * W_IMG], u8, kind="ExternalInput")
    out_d = nc.dram_tensor("out", [C_OUT, HS * W_IMG], bf, kind="ExternalOutput")

    with tile.TileContext(nc) as tc:
        with tc.tile_pool(name="const", bufs=1) as cpool, \
             tc.tile_pool(name="xp", bufs=1) as xpool, \
             tc.tile_pool(name="mp", bufs=3) as mpool, \
             tc.tile_pool(name="zp", bufs=4) as zpool, \
             tc.tile_pool(name="op", bufs=4) as opool, \
             tc.tile_pool(name="ps", bufs=8, space="PSUM") as ppool:

            # weights in three tiles so early matmuls wait only on the DMA
            # that carries their own stationary block (dependency waits are
            # tile-granular): w0 = block 0, wAr = blocks 1-2, wrest = 3-8
            w0 = cpool.tile([128, C_OUT], bf, name="w0", tag="w0")
            wAr = cpool.tile([128, 2 * C_OUT], bf, name="wAr", tag="wAr")
            wts = cpool.tile([128, 6 * C_OUT], bf, name="wts", tag="wts")
            bn = cpool.tile([C_OUT, 2], f32, name="bn_t", tag="bn_t")
            xs = xpool.tile([128, FREE], bf, name="xs_t", tag="xs_t")

            def load_x(p0, p1):
                nc.sync.dma_start(out=xs[:, p0 * WP:p1 * WP],
                                  in_=xs_d[:, p0 * WP:p1 * WP])

            mts = []

            def load_mask(m):
                mt = mpool.tile([C_OUT, G * W_IMG], u8, name="mt", tag="mt")
                nc.sync.dma_start(
                    out=mt[:], in_=mk_d[:, m * G * W_IMG:(m + 1) * G * W_IMG])
                mts.append(mt)

            # sync-ring FIFO: the first stationary block + fine-grained early
            # pairs land first so the PE starts fast; masks interleaved so
            # they arrive well before their group's DVE. The remaining weight
            # blocks + bn ride the scalar(ACT) ring in parallel (the scalar
            # ring carries nothing else until the first ACTIVATE).
            # PE clock priming: ~12 dummy matmuls on memset data keep the PE
            # busy during the initial DMA wait so the DVFS ramp completes
            # before the first real matmul
            dmem = cpool.tile([128, 256], bf, name="dmem", tag="dmem")
            nc.gpsimd.memset(dmem[:], 0.0)
            dps = ppool.tile([C_OUT, W_IMG], f32, name="dps", tag="pst")
            for _ in range(12):
                nc.tensor.matmul(dps[:, 0:256], dmem[:, 0:C_OUT], dmem[:],
                                 start=True, stop=True)

            # first-needed transfers split across both HWDGE rings, in the
            # order the PE consumes them:
            #   scalar: w0, wA1-2, pair1, bn, w3-8        sync: pair0, 2, 3...
            nc.scalar.dma_start(out=w0[:], in_=wp_d[:, 0:C_OUT])
            load_x(0, 1)
            nc.scalar.dma_start(out=wAr[:], in_=wp_d[:, C_OUT:3 * C_OUT])
            nc.scalar.dma_start(out=xs[:, WP:2 * WP], in_=xs_d[:, WP:2 * WP])
            load_x(2, 4)
            nc.scalar.dma_start(out=bn[:], in_=bn_d[:])
            nc.scalar.dma_start(out=wts[:], in_=wp_d[:, 3 * C_OUT:9 * C_OUT])
            load_mask(0)
            load_x(4, 8)
            load_x(8, 12)
            load_mask(1)
            load_x(12, 16)
            load_x(16, 20)
            load_mask(2)
            load_x(20, 24)
            load_x(24, 28)
            load_mask(3)
            load_x(28, NPAIR)
            for m in range(4, 8):
                load_mask(m)

            def epilogue(y, pst, ot):
                seg = slice((y % SG) * W_IMG, (y % SG + 1) * W_IMG)
                mt = mts[y // G]
                mseg = slice((y % G) * W_IMG, (y % G + 1) * W_IMG)
                if hw_lrelu:
                    nc.scalar.activation(
                        ot[:, seg], pst[:],
                        mybir.ActivationFunctionType.Lrelu,
                        bias=bn[:, 1:2], scale=bn[:, 0:1], alpha=LEAK)
                else:
                    zt = zpool.tile([C_OUT, W_IMG], f32, name="zt", tag="zt")
                    nc.scalar.activation(
                        zt[:], pst[:],
                        mybir.ActivationFunctionType.Identity,
                        bias=bn[:, 1:2], scale=bn[:, 0:1])
                    nc.vector.scalar_tensor_tensor(
                        ot[:, seg], zt[:], LEAK, zt[:],
                        op0=mybir.AluOpType.mult, op1=mybir.AluOpType.max)
                nc.vector.tensor_tensor(ot[:, seg], ot[:, seg], mt[:, mseg],
                                        op=mybir.AluOpType.mult)

            def w_ap(j):                      # full K=128 stationary tile j
                if j == 0:
                    return w0[:]
                if j < 3:
                    return wAr[:, (j - 1) * C_OUT:j * C_OUT]
                return wts[:, (j - 3) * C_OUT:(j - 2) * C_OUT]

            def mm(ps, w, off, start, stop):
                nc.tensor.matmul(ps[:], w, xs[:, off:off + W_IMG],
                                 start=start, stop=stop)

            def mm_h(ps, dx, lo, off, stop):  # K=64 lone-tap matmul
                rows = slice(0, 64) if lo else slice(64, 128)
                nc.tensor.matmul(ps[:], wts[rows, (3 + dx) * C_OUT:(4 + dx) * C_OUT],
                                 xs[rows, off:off + W_IMG],
                                 start=False, stop=stop)

            # 4 output rows per group s: y = 4s..4s+3, pairs t=2s..2s+2.
            #   wA[dx] (block dx):   even rows, pair t(+1): taps dy=0,1
            #   wB[dx] (block 3+dx): odd rows, pair t+1(+2): taps dy=1,2
            #   wC[dx] (block 6+dx): rows 0-63 dy=2 (even lone), 64-127 dy=0
            #     (odd lone); lone streams alternate PE row groups -> overlap.
            # Weight-major order: each stationary tile feeds 2 consecutive
            # matmuls so redundant LDWEIGHTS can be skipped.
            # Stores ride the sync ring: the scalar sequencer is near its
            # limit on ACTIVATEs alone and DMA_DIRECT2D issue costs ~590ns.
            for s in range(HS // SG - 1):
                t = 2 * s
                ot = opool.tile([C_OUT, SG * W_IMG], bf, name="ot", tag="ot")
                ps = [ppool.tile([C_OUT, W_IMG], f32, name=f"ps{i}", tag="pst")
                      for i in range(4)]
                for dx in range(3):
                    mm(ps[0], w_ap(dx), t * WP + dx, dx == 0, False)
                    mm(ps[2], w_ap(dx), (t + 1) * WP + dx, dx == 0, False)
                for dx in range(3):
                    mm(ps[1], w_ap(3 + dx), (t + 1) * WP + dx, dx == 0, False)
                    mm(ps[3], w_ap(3 + dx), (t + 2) * WP + dx, dx == 0, False)
                for dx in range(3):
                    mm_h(ps[0], dx, True, (t + 1) * WP + dx, dx == 2)
                    mm_h(ps[2], dx, True, (t + 2) * WP + dx, dx == 2)
                    mm_h(ps[1], dx, False, t * WP + dx, dx == 2)
                    mm_h(ps[3], dx, False, (t + 1) * WP + dx, dx == 2)
                for i in range(4):
                    epilogue(4 * s + i, ps[i], ot)
                d0 = s * SG * W_IMG
                nc.sync.dma_start(out=out_d[:, d0:d0 + SG * W_IMG], in_=ot[:])

            # last 4 rows: two 2-row paired subgroups (keeps the lone-tap PE
            # row-group overlap) with per-row stores on the otherwise-idle
            # scalar ring, so the post-matmul tail is short and the final
            # store doesn't queue behind earlier bulk stores on the sync ring
            for y0 in range(HS - SG, HS, 2):
                t = y0 // 2
                ps_a = ppool.tile([C_OUT, W_IMG], f32, name="ps_la", tag="pst")
                ps_b = ppool.tile([C_OUT, W_IMG], f32, name="ps_lb", tag="pst")
                for dx in range(3):
                    mm(ps_a, w_ap(dx), t * WP + dx, dx == 0, False)
                for dx in range(3):
                    mm(ps_b, w_ap(3 + dx), (t + 1) * WP + dx, dx == 0, False)
                for dx in range(3):
                    mm_h(ps_a, dx, True, (t + 1) * WP + dx, dx == 2)
                    mm_h(ps_b, dx, False, t * WP + dx, dx == 2)
                for y, pst in ((y0, ps_a), (y0 + 1, ps_b)):
                    ot = opool.tile([C_OUT, W_IMG], bf, name="otl", tag="otl")
                    mt = mts[y // G]
                    mseg = slice((y % G) * W_IMG, (y % G + 1) * W_IMG)
                    if hw_lrelu:
                        nc.scalar.activation(
                            ot[:], pst[:], mybir.ActivationFunctionType.Lrelu,
                            bias=bn[:, 1:2], scale=bn[:, 0:1], alpha=LEAK)
                    else:
                        zt = zpool.tile([C_OUT, W_IMG], f32, name="zt", tag="zt")
                        nc.scalar.activation(
                            zt[:], pst[:], mybir.ActivationFunctionType.Identity,
                            bias=bn[:, 1:2], scale=bn[:, 0:1])
                        nc.vector.scalar_tensor_tensor(
                            ot[:], zt[:], LEAK, zt[:],
                            op0=mybir.AluOpType.mult, op1=mybir.AluOpType.max)
                    nc.vector.tensor_tensor(ot[:], ot[:], mt[:, mseg],
                                            op=mybir.AluOpType.mult)
                    nc.scalar.dma_start(out=out_d[:, y * W_IMG:(y + 1) * W_IMG],
                                        in_=ot[:])
    nc.compile()
    return nc


def _get_program(hw_lrelu: bool = True) -> bass.Bass:
    key = ("nc", hw_lrelu)
    if key not in _CACHE:
        _CACHE[key] = _build_program(hw_lrelu)
    return _CACHE[key]


def make_in_maps(x, W, gamma, beta, mean, var, mask):
    """Host-side shard/pack of full inputs into per-core in_maps."""
    x = np.asarray(x, np.float32)
    W = np.asarray(W, np.float32)
    gamma = np.asarray(gamma, np.float32)
    beta = np.asarray(beta, np.float32)
    mean = np.asarray(mean, np.float32)
    var = np.asarray(var, np.float32)
    mask = np.asarray(mask)

    xp = np.pad(x[0], ((0, 0), (1, 1), (1, 1)), mode="reflect")   # [64,514,514]
    xpb = xp.astype(bf16)

    # 9 stationary blocks [K=ci, M=co]: see _build_program docstring
    wt = W.transpose(1, 0, 2, 3).astype(np.float32)               # [ci,co,dy,dx]
    wp = np.zeros((128, 9 * C_OUT), np.float32)
    for dx in range(3):
        wp[0:64, dx * C_OUT:(dx + 1) * C_OUT] = wt[:, :, 0, dx]
        wp[64:128, dx * C_OUT:(dx + 1) * C_OUT] = wt[:, :, 1, dx]
        wp[0:64, (3 + dx) * C_OUT:(4 + dx) * C_OUT] = wt[:, :, 1, dx]
        wp[64:128, (3 + dx) * C_OUT:(4 + dx) * C_OUT] = wt[:, :, 2, dx]
        wp[0:64, (6 + dx) * C_OUT:(7 + dx) * C_OUT] = wt[:, :, 2, dx]
        wp[64:128, (6 + dx) * C_OUT:(7 + dx) * C_OUT] = wt[:, :, 0, dx]
    wp = wp.astype(bf16)

    inv = 1.0 / np.sqrt(var + EPS)
    bn = np.stack([gamma * inv, beta - mean * gamma * inv],
                  axis=1).astype(np.float32)                      # [128,2]

    m8 = mask[0].astype(np.uint8)                                 # [128,512,512]

    in_maps = []
    for c in range(N_CORES):
        S = xpb[:, HS * c:HS * c + HS + 2, :]                     # 66 rows
        even = np.ascontiguousarray(S[:, 0::2, :]).reshape(C_IN, FREE)
        odd = np.ascontiguousarray(S[:, 1::2, :]).reshape(C_IN, FREE)
        xs_c = np.concatenate([even, odd], axis=0)                # [128, FREE]
        mk_c = np.ascontiguousarray(
            m8[:, HS * c:HS * c + HS, :]).reshape(C_OUT, HS * W_IMG)
        in_maps.append(dict(xs=xs_c, wp=wp, bn=bn, msk=mk_c))
    return in_maps


def kernel(x, W, gamma, beta, mean, var, mask, _trace=False):
    global LAST_RESULTS
    nc = _get_program()
    in_maps = make_in_maps(x, W, gamma, beta, mean, var, mask)
    res = run_bass_kernel_spmd(nc, in_maps, list(range(N_CORES)), trace=_trace)
    LAST_RESULTS = res
    out = np.empty((1, C_OUT, H, W_IMG), np.float32)
    for c in range(N_CORES):
        out[0, :, HS * c:HS * c + HS, :] = \
            np.asarray(res.results[c]["out"]).astype(np.float32) \
              .reshape(C_OUT, HS, W_IMG)
    return out


# revision 16
# speedup vs baseline: 1.0317x; 1.0317x over previous
"""Trainium2 Bass kernel for: 3x3 conv (reflect pad) + BatchNorm + LeakyReLU + mask.

Input  x:    (1, 64, 512, 512) f32
       W:    (128, 64, 3, 3)   f32
       gamma/beta/mean/var: (128,) f32
       mask: (1, 128, 512, 512) int32 (0/1)
Output (1, 128, 512, 512) f32

Strategy (8 cores, SPMD):
  - Shard H spatially: core c computes output rows [64c, 64c+64).
  - Even/odd row interleave, single x copy: host reflect-pads x to
    (64, 514, 514); core c takes its 66-row slab and ships it ONCE as a
    [128, 33*514] bf16 image: partitions 0..63 hold channel ci's EVEN local
    rows (pair index p -> row 2p), partitions 64..127 hold the ODD rows
    (p -> row 2p+1). A K=128 matmul at pair offset p then contracts over two
    adjacent image rows at once (two conv dy taps in one slot).
  - Output row y=2t: pair t covers taps dy=0,1; the lone dy=2 tap (even row
    2t+2) runs as a K=64 matmul on PE rows 0-63. Row y=2t+1: pair t+1 covers
    dy=1,2; lone dy=0 (odd row 2t+1) on PE rows 64-127. The two lone streams
    use disjoint PE row groups -> concurrent, so 2 rows cost 9 matmul slots
    (the algorithmic minimum for 9 taps at K=64 on a K=128 array).
  - 4-row groups, weight-major matmul order: consecutive matmuls reuse the
    same stationary tile so the PE can skip redundant LDWEIGHTS.
  - Epilogue: ACT Lrelu(psum*scale+shift) -> bf16, DVE multiply by uint8
    mask; bf16 stores (harness tolerance 2e-2 >> bf16 rounding).
  - DMA: x+masks on the sync HWDGE ring, weights first + stores on the
    scalar HWDGE ring. No SWDGE.
"""

import numpy as np
import ml_dtypes

import concourse.bacc as bacc
import concourse.bass as bass
import concourse.mybir as mybir
import concourse.tile as tile
from concourse.bass_utils import run_bass_kernel_spmd

bf16 = ml_dtypes.bfloat16

N_CORES = 8
C_IN = 64
C_OUT = 128
H = 512
W_IMG = 512
HS = H // N_CORES            # 64 output rows per core
WP = W_IMG + 2               # 514 padded columns
NPAIR = HS // 2 + 1          # 33 even/odd row pairs per core
FREE = NPAIR * WP            # per-partition free elems of the x image
G = 8                        # output rows per mask tile
SG = 4                       # output rows per store tile / PSUM group
LEAK = 0.01
EPS = 1e-5

_CACHE = {}
LAST_RESULTS = None          # BassKernelResults of the last run (for test.py)


def _build_program(hw_lrelu: bool = True) -> bass.Bass:
    """hw_lrelu=True uses the ACT engine's native Lrelu (not implemented in
    CoreSim); False uses an Identity + DVE max(z*a, z) fallback."""
    nc = bacc.Bacc("TRN2", target_bir_lowering=False, debug=False,
                   num_devices=N_CORES)
    f32 = mybir.dt.float32
    bf = mybir.dt.bfloat16
    u8 = mybir.dt.uint8

    xs_d = nc.dram_tensor("xs", [128, FREE], bf, kind="ExternalInput")
    wp_d = nc.dram_tensor("wp", [128, 9 * C_OUT], bf, kind="ExternalInput")
    bn_d = nc.dram_tensor("bn", [C_OUT, 2], f32, kind="ExternalInput")
    mk_d = nc.dram_tensor("msk", [C_OUT, HS * W_IMG], u8, kind="ExternalInput")
    out_d = nc.dram_tensor("out", [C_OUT, HS * W_IMG], bf, kind="ExternalOutput")

    with tile.TileContext(nc) as tc:
        with tc.tile_pool(name="const", bufs=1) as cpool, \
             tc.tile_pool(name="xp", bufs=1) as xpool, \
             tc.tile_pool(name="mp", bufs=3) as mpool, \
             tc.tile_pool(name="zp", bufs=4) as zpool, \
             tc.tile_pool(name="op", bufs=4) as opool, \
             tc.tile_pool(name="ps", bufs=8, space="PSUM") as ppool:

            # weights in three tiles so early matmuls wait only on the DMA
            # that carries their own stationary block (dependency waits are
            # tile-granular): w0 = block 0, wAr = blocks 1-2, wrest = 3-8
            w0 = cpool.tile([128, C_OUT], bf, name="w0", tag="w0")
            wAr = cpool.tile([128, 2 * C_OUT], bf, name="wAr", tag="wAr")
            wts = cpool.tile([128, 6 * C_OUT], bf, name="wts", tag="wts")
            bn = cpool.tile([C_OUT, 2], f32, name="bn_t", tag="bn_t")
            xs = xpool.tile([128, FREE], bf, name="xs_t", tag="xs_t")

            def load_x(p0, p1):
                nc.sync.dma_start(out=xs[:, p0 * WP:p1 * WP],
                                  in_=xs_d[:, p0 * WP:p1 * WP])

            mts = []

            def load_mask(m):
                mt = mpool.tile([C_OUT, G * W_IMG], u8, name="mt", tag="mt")
                nc.sync.dma_start(
                    out=mt[:], in_=mk_d[:, m * G * W_IMG:(m + 1) * G * W_IMG])
                mts.append(mt)

            # sync-ring FIFO: the first stationary block + fine-grained early
            # pairs land first so the PE starts fast; masks interleaved so
            # they arrive well before their group's DVE. The remaining weight
            # blocks + bn ride the scalar(ACT) ring in parallel (the scalar
            # ring carries nothing else until the first ACTIVATE).
            # PE clock priming: ~12 dummy matmuls on memset data keep the PE
            # busy during the initial DMA wait so the DVFS ramp completes
            # before the first real matmul
            dmem = cpool.tile([128, 256], bf, name="dmem", tag="dmem")
            nc.gpsimd.memset(dmem[:], 0.0)
            dps = ppool.tile([C_OUT, W_IMG], f32, name="dps", tag="pst")
            for _ in range(8):
                nc.tensor.matmul(dps[:, 0:256], dmem[:, 0:C_OUT], dmem[:],
                                 start=True, stop=True)

            # scalar ring: weights only (small descriptors run at half DMA
            # rate — never put x pairs behind them). sync ring: x pairs at
            # full rate, finest chunks first.
            nc.scalar.dma_start(out=w0[:], in_=wp_d[:, 0:C_OUT])
            nc.scalar.dma_start(out=wAr[:], in_=wp_d[:, C_OUT:3 * C_OUT])
            nc.scalar.dma_start(out=wts[:], in_=wp_d[:, 3 * C_OUT:9 * C_OUT])
            nc.scalar.dma_start(out=bn[:], in_=bn_d[:])
            load_x(0, 2)
            load_x(2, 4)
            load_mask(0)
            load_x(4, 8)
            load_x(8, 12)
            load_mask(1)
            load_x(12, 16)
            load_x(16, 20)
            load_mask(2)
            load_x(20, 24)
            load_x(24, 28)
            load_mask(3)
            load_x(28, NPAIR)
            for m in range(4, 8):
                load_mask(m)

            def epilogue(y, pst, ot):
                seg = slice((y % SG) * W_IMG, (y % SG + 1) * W_IMG)
                mt = mts[y // G]
                mseg = slice((y % G) * W_IMG, (y % G + 1) * W_IMG)
                if hw_lrelu:
                    nc.scalar.activation(
                        ot[:, seg], pst[:],
                        mybir.ActivationFunctionType.Lrelu,
                        bias=bn[:, 1:2], scale=bn[:, 0:1], alpha=LEAK)
                else:
                    zt = zpool.tile([C_OUT, W_IMG], f32, name="zt", tag="zt")
                    nc.scalar.activation(
                        zt[:], pst[:],
                        mybir.ActivationFunctionType.Identity,
                        bias=bn[:, 1:2], scale=bn[:, 0:1])
                    nc.vector.scalar_tensor_tensor(
                        ot[:, seg], zt[:], LEAK, zt[:],
                        op0=mybir.AluOpType.mult, op1=mybir.AluOpType.max)
                nc.vector.tensor_tensor(ot[:, seg], ot[:, seg], mt[:, mseg],
                                        op=mybir.AluOpType.mult)

            def w_ap(j):                      # full K=128 stationary tile j
                if j == 0:
                    return w0[:]
                if j < 3:
                    return wAr[:, (j - 1) * C_OUT:j * C_OUT]
                return wts[:, (j - 3) * C_OUT:(j - 2) * C_OUT]

            def mm(ps, w, off, start, stop):
                nc.tensor.matmul(ps[:], w, xs[:, off:off + W_IMG],
                                 start=start, stop=stop)

            def mm_h(ps, dx, lo, off, stop):  # K=64 lone-tap matmul
                rows = slice(0, 64) if lo else slice(64, 128)
                nc.tensor.matmul(ps[:], wts[rows, (3 + dx) * C_OUT:(4 + dx) * C_OUT],
                                 xs[rows, off:off + W_IMG],
                                 start=False, stop=stop)

            # 4 output rows per group s: y = 4s..4s+3, pairs t=2s..2s+2.
            #   wA[dx] (block dx):   even rows, pair t(+1): taps dy=0,1
            #   wB[dx] (block 3+dx): odd rows, pair t+1(+2): taps dy=1,2
            #   wC[dx] (block 6+dx): rows 0-63 dy=2 (even lone), 64-127 dy=0
            #     (odd lone); lone streams alternate PE row groups -> overlap.
            # Weight-major order: each stationary tile feeds 2 consecutive
            # matmuls so redundant LDWEIGHTS can be skipped.
            # Stores ride the sync ring: the scalar sequencer is near its
            # limit on ACTIVATEs alone and DMA_DIRECT2D issue costs ~590ns.
            for s in range(HS // SG - 1):
                t = 2 * s
                ot = opool.tile([C_OUT, SG * W_IMG], bf, name="ot", tag="ot")
                ps = [ppool.tile([C_OUT, W_IMG], f32, name=f"ps{i}", tag="pst")
                      for i in range(4)]
                for dx in range(3):
                    mm(ps[0], w_ap(dx), t * WP + dx, dx == 0, False)
                    mm(ps[2], w_ap(dx), (t + 1) * WP + dx, dx == 0, False)
                for dx in range(3):
                    mm(ps[1], w_ap(3 + dx), (t + 1) * WP + dx, dx == 0, False)
                    mm(ps[3], w_ap(3 + dx), (t + 2) * WP + dx, dx == 0, False)
                for dx in range(3):
                    mm_h(ps[0], dx, True, (t + 1) * WP + dx, dx == 2)
                    mm_h(ps[2], dx, True, (t + 2) * WP + dx, dx == 2)
                    mm_h(ps[1], dx, False, t * WP + dx, dx == 2)
                    mm_h(ps[3], dx, False, (t + 1) * WP + dx, dx == 2)
                for i in range(4):
                    epilogue(4 * s + i, ps[i], ot)
                d0 = s * SG * W_IMG
                nc.sync.dma_start(out=out_d[:, d0:d0 + SG * W_IMG], in_=ot[:])

            # last 4 rows: two 2-row paired subgroups (keeps the lone-tap PE
            # row-group overlap) with per-row stores on the otherwise-idle
            # scalar ring, so the post-matmul tail is short and the final
            # store doesn't queue behind earlier bulk stores on the sync ring
            for y0 in range(HS - SG, HS, 2):
                t = y0 // 2
                ps_a = ppool.tile([C_OUT, W_IMG], f32, name="ps_la", tag="pst")
                ps_b = ppool.tile([C_OUT, W_IMG], f32, name="ps_lb", tag="pst")
                for dx in range(3):
                    mm(ps_a, w_ap(dx), t * WP + dx, dx == 0, False)
                for dx in range(3):
                    mm(ps_b, w_ap(3 + dx), (t + 1) * WP + dx, dx == 0, False)
                for dx in range(3):
                    mm_h(ps_a, dx, True, (t + 1) * WP + dx, dx == 2)
                    mm_h(ps_b, dx, False, t * WP + dx, dx == 2)
                for y, pst in ((y0, ps_a), (y0 + 1, ps_b)):
                    ot = opool.tile([C_OUT, W_IMG], bf, name="otl", tag="otl")
                    mt = mts[y // G]
                    mseg = slice((y % G) * W_IMG, (y % G + 1) * W_IMG)
                    if hw_lrelu:
                        nc.scalar.activation(
                            ot[:], pst[:], mybir.ActivationFunctionType.Lrelu,
                            bias=bn[:, 1:2], scale=bn[:, 0:1], alpha=LEAK)
                    else:
                        zt = zpool.tile([C_OUT, W_IMG], f32, name="zt", tag="zt")
                        nc.scalar.activation(
                            zt[:], pst[:], mybir.ActivationFunctionType.Identity,
                            bias=bn[:, 1:2], scale=bn[:, 0:1])
                        nc.vector.scalar_tensor_tensor(
                            ot[:], zt[:], LEAK, zt[:],
                            op0=mybir.AluOpType.mult, op1=mybir.AluOpType.max)
                    nc.vector.tensor_tensor(ot[:], ot[:], mt[:, mseg],
                                            op=mybir.AluOpType.mult)
                    nc.scalar.dma_start(out=out_d[:, y * W_IMG:(y + 1) * W_IMG],
                                        in_=ot[:])
    nc.compile()
    return nc


def _get_program(hw_lrelu: bool = True) -> bass.Bass:
    key = ("nc", hw_lrelu)
    if key not in _CACHE:
        _CACHE[key] = _build_program(hw_lrelu)
    return _CACHE[key]


def make_in_maps(x, W, gamma, beta, mean, var, mask):
    """Host-side shard/pack of full inputs into per-core in_maps."""
    x = np.asarray(x, np.float32)
    W = np.asarray(W, np.float32)
    gamma = np.asarray(gamma, np.float32)
    beta = np.asarray(beta, np.float32)
    mean = np.asarray(mean, np.float32)
    var = np.asarray(var, np.float32)
    mask = np.asarray(mask)

    xp = np.pad(x[0], ((0, 0), (1, 1), (1, 1)), mode="reflect")   # [64,514,514]
    xpb = xp.astype(bf16)

    # 9 stationary blocks [K=ci, M=co]: see _build_program docstring
    wt = W.transpose(1, 0, 2, 3).astype(np.float32)               # [ci,co,dy,dx]
    wp = np.zeros((128, 9 * C_OUT), np.float32)
    for dx in range(3):
        wp[0:64, dx * C_OUT:(dx + 1) * C_OUT] = wt[:, :, 0, dx]
        wp[64:128, dx * C_OUT:(dx + 1) * C_OUT] = wt[:, :, 1, dx]
        wp[0:64, (3 + dx) * C_OUT:(4 + dx) * C_OUT] = wt[:, :, 1, dx]
        wp[64:128, (3 + dx) * C_OUT:(4 + dx) * C_OUT] = wt[:, :, 2, dx]
        wp[0:64, (6 + dx) * C_OUT:(7 + dx) * C_OUT] = wt[:, :, 2, dx]
        wp[64:128, (6 + dx) * C_OUT:(7 + dx) * C_OUT] = wt[:, :, 0, dx]
    wp = wp.astype(bf16)

    inv = 1.0 / np.sqrt(var + EPS)
    bn = np.stack([gamma * inv, beta - mean * gamma * inv],
                  axis=1).astype(np.float32)                      # [128,2]

    m8 = mask[0].astype(np.uint8)                                 # [128,512,512]

    in_maps = []
    for c in range(N_CORES):
        S = xpb[:, HS * c:HS * c + HS + 2, :]                     # 66 rows
        even = np.ascontiguousarray(S[:, 0::2, :]).reshape(C_IN, FREE)
        odd = np.ascontiguousarray(S[:, 1::2, :]).reshape(C_IN, FREE)
        xs_c = np.concatenate([even, odd], axis=0)                # [128, FREE]
        mk_c = np.ascontiguousarray(
            m8[:, HS * c:HS * c + HS, :]).reshape(C_OUT, HS * W_IMG)
        in_maps.append(dict(xs=xs_c, wp=wp, bn=bn, msk=mk_c))
    return in_maps


def kernel(x, W, gamma, beta, mean, var, mask, _trace=False):
    global LAST_RESULTS
    nc = _get_program()
    in_maps = make_in_maps(x, W, gamma, beta, mean, var, mask)
    res = run_bass_kernel_spmd(nc, in_maps, list(range(N_CORES)), trace=_trace)
    LAST_RESULTS = res
    out = np.empty((1, C_OUT, H, W_IMG), np.float32)
    for c in range(N_CORES):
        out[0, :, HS * c:HS * c + HS, :] = \
            np.asarray(res.results[c]["out"]).astype(np.float32) \
              .reshape(C_OUT, HS, W_IMG)
    return out


# revision 18
# speedup vs baseline: 1.0554x; 1.0230x over previous
"""Trainium2 Bass kernel for: 3x3 conv (reflect pad) + BatchNorm + LeakyReLU + mask.

Input  x:    (1, 64, 512, 512) f32
       W:    (128, 64, 3, 3)   f32
       gamma/beta/mean/var: (128,) f32
       mask: (1, 128, 512, 512) int32 (0/1)
Output (1, 128, 512, 512) f32

Strategy (8 cores, SPMD):
  - Shard H spatially: core c computes output rows [64c, 64c+64).
  - Even/odd row interleave, single x copy: host reflect-pads x to
    (64, 514, 514); core c takes its 66-row slab and ships it ONCE as a
    [128, 33*514] bf16 image: partitions 0..63 hold channel ci's EVEN local
    rows (pair index p -> row 2p), partitions 64..127 hold the ODD rows
    (p -> row 2p+1). A K=128 matmul at pair offset p then contracts over two
    adjacent image rows at once (two conv dy taps in one slot).
  - Output row y=2t: pair t covers taps dy=0,1; the lone dy=2 tap (even row
    2t+2) runs as a K=64 matmul on PE rows 0-63. Row y=2t+1: pair t+1 covers
    dy=1,2; lone dy=0 (odd row 2t+1) on PE rows 64-127. The two lone streams
    use disjoint PE row groups -> concurrent, so 2 rows cost 9 matmul slots
    (the algorithmic minimum for 9 taps at K=64 on a K=128 array).
  - 4-row groups, weight-major matmul order: consecutive matmuls reuse the
    same stationary tile so the PE can skip redundant LDWEIGHTS.
  - Epilogue: ACT Lrelu(psum*scale+shift) -> bf16, DVE multiply by uint8
    mask; bf16 stores (harness tolerance 2e-2 >> bf16 rounding).
  - DMA: x+masks on the sync HWDGE ring, weights first + stores on the
    scalar HWDGE ring. No SWDGE.
"""

import numpy as np
import ml_dtypes

import concourse.bacc as bacc
import concourse.bass as bass
import concourse.mybir as mybir
import concourse.tile as tile
from concourse.bass_utils import run_bass_kernel_spmd

bf16 = ml_dtypes.bfloat16

N_CORES = 8
C_IN = 64
C_OUT = 128
H = 512
W_IMG = 512
HS = H // N_CORES            # 64 output rows per core
WP = W_IMG + 2               # 514 padded columns
NPAIR = HS // 2 + 1          # 33 even/odd row pairs per core
FREE = NPAIR * WP            # per-partition free elems of the x image
G = 8                        # output rows per mask tile
SG = 4                       # output rows per store tile / PSUM group
LEAK = 0.01
EPS = 1e-5

_CACHE = {}
LAST_RESULTS = None          # BassKernelResults of the last run (for test.py)


def _build_program(hw_lrelu: bool = True) -> bass.Bass:
    """hw_lrelu=True uses the ACT engine's native Lrelu (not implemented in
    CoreSim); False uses an Identity + DVE max(z*a, z) fallback."""
    nc = bacc.Bacc("TRN2", target_bir_lowering=False, debug=False,
                   num_devices=N_CORES)
    f32 = mybir.dt.float32
    bf = mybir.dt.bfloat16
    u8 = mybir.dt.uint8

    xs_d = nc.dram_tensor("xs", [128, FREE], bf, kind="ExternalInput")
    wp_d = nc.dram_tensor("wp", [128, 9 * C_OUT], bf, kind="ExternalInput")
    bn_d = nc.dram_tensor("bn", [C_OUT, 2], f32, kind="ExternalInput")
    mk_d = nc.dram_tensor("msk", [C_OUT, HS * W_IMG], u8, kind="ExternalInput")
    out_d = nc.dram_tensor("out", [C_OUT, HS * W_IMG], bf, kind="ExternalOutput")

    with tile.TileContext(nc) as tc:
        with tc.tile_pool(name="const", bufs=1) as cpool, \
             tc.tile_pool(name="xp", bufs=1) as xpool, \
             tc.tile_pool(name="mp", bufs=3) as mpool, \
             tc.tile_pool(name="zp", bufs=4) as zpool, \
             tc.tile_pool(name="op", bufs=4) as opool, \
             tc.tile_pool(name="ps", bufs=8, space="PSUM") as ppool:

            # weights in three tiles so early matmuls wait only on the DMA
            # that carries their own stationary block (dependency waits are
            # tile-granular): w0 = block 0, wAr = blocks 1-2, wrest = 3-8
            w0 = cpool.tile([128, C_OUT], bf, name="w0", tag="w0")
            wAr = cpool.tile([128, 2 * C_OUT], bf, name="wAr", tag="wAr")
            wts = cpool.tile([128, 6 * C_OUT], bf, name="wts", tag="wts")
            bn = cpool.tile([C_OUT, 2], f32, name="bn_t", tag="bn_t")
            xs = xpool.tile([128, FREE], bf, name="xs_t", tag="xs_t")

            def load_x(p0, p1):
                nc.sync.dma_start(out=xs[:, p0 * WP:p1 * WP],
                                  in_=xs_d[:, p0 * WP:p1 * WP])

            mts = []

            def load_mask(m):
                mt = mpool.tile([C_OUT, G * W_IMG], u8, name="mt", tag="mt")
                nc.sync.dma_start(
                    out=mt[:], in_=mk_d[:, m * G * W_IMG:(m + 1) * G * W_IMG])
                mts.append(mt)

            # sync-ring FIFO: the first stationary block + fine-grained early
            # pairs land first so the PE starts fast; masks interleaved so
            # they arrive well before their group's DVE. The remaining weight
            # blocks + bn ride the scalar(ACT) ring in parallel (the scalar
            # ring carries nothing else until the first ACTIVATE).
            # scalar ring: weights only (small descriptors run at half DMA
            # rate — never put x pairs behind them). sync ring: x pairs at
            # full rate, finest chunks first so the PE starts early and
            # never stutters (each PE stall resets the slow DVFS ramp).
            nc.scalar.dma_start(out=w0[:], in_=wp_d[:, 0:C_OUT])
            nc.scalar.dma_start(out=wAr[:], in_=wp_d[:, C_OUT:3 * C_OUT])
            nc.scalar.dma_start(out=wts[:], in_=wp_d[:, 3 * C_OUT:9 * C_OUT])
            nc.scalar.dma_start(out=bn[:], in_=bn_d[:])
            for p in range(4):
                load_x(p, p + 1)
            load_mask(0)
            load_x(4, 8)
            load_x(8, 12)
            load_mask(1)
            load_x(12, 16)
            load_x(16, 20)
            load_mask(2)
            load_x(20, 24)
            load_x(24, 28)
            load_mask(3)
            load_x(28, NPAIR)
            for m in range(4, 8):
                load_mask(m)

            def epilogue(y, pst, ot):
                seg = slice((y % SG) * W_IMG, (y % SG + 1) * W_IMG)
                mt = mts[y // G]
                mseg = slice((y % G) * W_IMG, (y % G + 1) * W_IMG)
                if hw_lrelu:
                    nc.scalar.activation(
                        ot[:, seg], pst[:],
                        mybir.ActivationFunctionType.Lrelu,
                        bias=bn[:, 1:2], scale=bn[:, 0:1], alpha=LEAK)
                else:
                    zt = zpool.tile([C_OUT, W_IMG], f32, name="zt", tag="zt")
                    nc.scalar.activation(
                        zt[:], pst[:],
                        mybir.ActivationFunctionType.Identity,
                        bias=bn[:, 1:2], scale=bn[:, 0:1])
                    nc.vector.scalar_tensor_tensor(
                        ot[:, seg], zt[:], LEAK, zt[:],
                        op0=mybir.AluOpType.mult, op1=mybir.AluOpType.max)
                nc.vector.tensor_tensor(ot[:, seg], ot[:, seg], mt[:, mseg],
                                        op=mybir.AluOpType.mult)

            def w_ap(j):                      # full K=128 stationary tile j
                if j == 0:
                    return w0[:]
                if j < 3:
                    return wAr[:, (j - 1) * C_OUT:j * C_OUT]
                return wts[:, (j - 3) * C_OUT:(j - 2) * C_OUT]

            def mm(ps, w, off, start, stop):
                nc.tensor.matmul(ps[:], w, xs[:, off:off + W_IMG],
                                 start=start, stop=stop)

            def mm_h(ps, dx, lo, off, stop):  # K=64 lone-tap matmul
                rows = slice(0, 64) if lo else slice(64, 128)
                nc.tensor.matmul(ps[:], wts[rows, (3 + dx) * C_OUT:(4 + dx) * C_OUT],
                                 xs[rows, off:off + W_IMG],
                                 start=False, stop=stop)

            # 4 output rows per group s: y = 4s..4s+3, pairs t=2s..2s+2.
            #   wA[dx] (block dx):   even rows, pair t(+1): taps dy=0,1
            #   wB[dx] (block 3+dx): odd rows, pair t+1(+2): taps dy=1,2
            #   wC[dx] (block 6+dx): rows 0-63 dy=2 (even lone), 64-127 dy=0
            #     (odd lone); lone streams alternate PE row groups -> overlap.
            # Weight-major order: each stationary tile feeds 2 consecutive
            # matmuls so redundant LDWEIGHTS can be skipped.
            # Stores ride the sync ring: the scalar sequencer is near its
            # limit on ACTIVATEs alone and DMA_DIRECT2D issue costs ~590ns.
            for s in range(HS // SG - 1):
                t = 2 * s
                ot = opool.tile([C_OUT, SG * W_IMG], bf, name="ot", tag="ot")
                ps = [ppool.tile([C_OUT, W_IMG], f32, name=f"ps{i}", tag="pst")
                      for i in range(4)]
                for dx in range(3):
                    mm(ps[0], w_ap(dx), t * WP + dx, dx == 0, False)
                    mm(ps[2], w_ap(dx), (t + 1) * WP + dx, dx == 0, False)
                for dx in range(3):
                    mm(ps[1], w_ap(3 + dx), (t + 1) * WP + dx, dx == 0, False)
                    mm(ps[3], w_ap(3 + dx), (t + 2) * WP + dx, dx == 0, False)
                for dx in range(3):
                    mm_h(ps[0], dx, True, (t + 1) * WP + dx, dx == 2)
                    mm_h(ps[2], dx, True, (t + 2) * WP + dx, dx == 2)
                    mm_h(ps[1], dx, False, t * WP + dx, dx == 2)
                    mm_h(ps[3], dx, False, (t + 1) * WP + dx, dx == 2)
                for i in range(4):
                    epilogue(4 * s + i, ps[i], ot)
                d0 = s * SG * W_IMG
                nc.sync.dma_start(out=out_d[:, d0:d0 + SG * W_IMG], in_=ot[:])

            # last 4 rows: two 2-row paired subgroups (keeps the lone-tap PE
            # row-group overlap) with per-row stores on the otherwise-idle
            # scalar ring, so the post-matmul tail is short and the final
            # store doesn't queue behind earlier bulk stores on the sync ring
            for y0 in range(HS - SG, HS, 2):
                t = y0 // 2
                ps_a = ppool.tile([C_OUT, W_IMG], f32, name="ps_la", tag="pst")
                ps_b = ppool.tile([C_OUT, W_IMG], f32, name="ps_lb", tag="pst")
                for dx in range(3):
                    mm(ps_a, w_ap(dx), t * WP + dx, dx == 0, False)
                for dx in range(3):
                    mm(ps_b, w_ap(3 + dx), (t + 1) * WP + dx, dx == 0, False)
                for dx in range(3):
                    mm_h(ps_a, dx, True, (t + 1) * WP + dx, dx == 2)
                    mm_h(ps_b, dx, False, t * WP + dx, dx == 2)
                for y, pst in ((y0, ps_a), (y0 + 1, ps_b)):
                    # the very last row goes in half-row pieces so the final
                    # ACT -> DVE -> store chain after the last matmul is short
                    halves = 2 if y == HS - 1 else 1
                    hw_ = W_IMG // halves
                    ot = opool.tile([C_OUT, W_IMG], bf, name="otl", tag="otl")
                    mt = mts[y // G]
                    for h in range(halves):
                        hseg = slice(h * hw_, (h + 1) * hw_)
                        mseg = slice((y % G) * W_IMG + h * hw_,
                                     (y % G) * W_IMG + (h + 1) * hw_)
                        if hw_lrelu:
                            nc.scalar.activation(
                                ot[:, hseg], pst[:, hseg],
                                mybir.ActivationFunctionType.Lrelu,
                                bias=bn[:, 1:2], scale=bn[:, 0:1], alpha=LEAK)
                        else:
                            zt = zpool.tile([C_OUT, W_IMG], f32,
                                            name="zt", tag="zt")
                            nc.scalar.activation(
                                zt[:, hseg], pst[:, hseg],
                                mybir.ActivationFunctionType.Identity,
                                bias=bn[:, 1:2], scale=bn[:, 0:1])
                            nc.vector.scalar_tensor_tensor(
                                ot[:, hseg], zt[:, hseg], LEAK, zt[:, hseg],
                                op0=mybir.AluOpType.mult,
                                op1=mybir.AluOpType.max)
                        nc.vector.tensor_tensor(ot[:, hseg], ot[:, hseg],
                                                mt[:, mseg],
                                                op=mybir.AluOpType.mult)
                        nc.scalar.dma_start(
                            out=out_d[:, y * W_IMG + h * hw_:
                                      y * W_IMG + (h + 1) * hw_],
                            in_=ot[:, hseg])
    nc.compile()
    return nc


def _get_program(hw_lrelu: bool = True) -> bass.Bass:
    key = ("nc", hw_lrelu)
    if key not in _CACHE:
        _CACHE[key] = _build_program(hw_lrelu)
    return _CACHE[key]


def make_in_maps(x, W, gamma, beta, mean, var, mask):
    """Host-side shard/pack of full inputs into per-core in_maps."""
    x = np.asarray(x, np.float32)
    W = np.asarray(W, np.float32)
    gamma = np.asarray(gamma, np.float32)
    beta = np.asarray(beta, np.float32)
    mean = np.asarray(mean, np.float32)
    var = np.asarray(var, np.float32)
    mask = np.asarray(mask)

    xp = np.pad(x[0], ((0, 0), (1, 1), (1, 1)), mode="reflect")   # [64,514,514]
    xpb = xp.astype(bf16)

    # 9 stationary blocks [K=ci, M=co]: see _build_program docstring
    wt = W.transpose(1, 0, 2, 3).astype(np.float32)               # [ci,co,dy,dx]
    wp = np.zeros((128, 9 * C_OUT), np.float32)
    for dx in range(3):
        wp[0:64, dx * C_OUT:(dx + 1) * C_OUT] = wt[:, :, 0, dx]
        wp[64:128, dx * C_OUT:(dx + 1) * C_OUT] = wt[:, :, 1, dx]
        wp[0:64, (3 + dx) * C_OUT:(4 + dx) * C_OUT] = wt[:, :, 1, dx]
        wp[64:128, (3 + dx) * C_OUT:(4 + dx) * C_OUT] = wt[:, :, 2, dx]
        wp[0:64, (6 + dx) * C_OUT:(7 + dx) * C_OUT] = wt[:, :, 2, dx]
        wp[64:128, (6 + dx) * C_OUT:(7 + dx) * C_OUT] = wt[:, :, 0, dx]
    wp = wp.astype(bf16)

    inv = 1.0 / np.sqrt(var + EPS)
    bn = np.stack([gamma * inv, beta - mean * gamma * inv],
                  axis=1).astype(np.float32)                      # [128,2]

    m8 = mask[0].astype(np.uint8)                                 # [128,512,512]

    in_maps = []
    for c in range(N_CORES):
        S = xpb[:, HS * c:HS * c + HS + 2, :]                     # 66 rows
        even = np.ascontiguousarray(S[:, 0::2, :]).reshape(C_IN, FREE)
        odd = np.ascontiguousarray(S[:, 1::2, :]).reshape(C_IN, FREE)
        xs_c = np.concatenate([even, odd], axis=0)                # [128, FREE]
        mk_c = np.ascontiguousarray(
            m8[:, HS * c:HS * c + HS, :]).reshape(C_OUT, HS * W_IMG)
        in_maps.append(dict(xs=xs_c, wp=wp, bn=bn, msk=mk_c))
    return in_maps


def kernel(x, W, gamma, beta, mean, var, mask, _trace=False):
    global LAST_RESULTS
    nc = _get_program()
    in_maps = make_in_maps(x, W, gamma, beta, mean, var, mask)
    res = run_bass_kernel_spmd(nc, in_maps, list(range(N_CORES)), trace=_trace)
    LAST_RESULTS = res
    out = np.empty((1, C_OUT, H, W_IMG), np.float32)
    for c in range(N_CORES):
        out[0, :, HS * c:HS * c + HS, :] = \
            np.asarray(res.results[c]["out"]).astype(np.float32) \
              .reshape(C_OUT, HS, W_IMG)
    return out
